# revision 35
# baseline (speedup 1.0000x reference)
"""Trainium2 Bass kernel for nn_HEALDownSampler (gnn_message_passing).

Reference computation:
    e   = gelu(edge_attr @ we1 + be1) @ we2 + be2            # [E, 64]
    vm  = concat([broadcast(e), x], -1)                      # [B, E, 192]
    agg = segment_sum(vm, edge_rec, R)                       # [B, R, 192]
    out = gelu(agg @ wf1 + bf1) @ wf2 + bf2                  # [B, R, 128]

Key algebraic restructuring:
    agg @ wf1 = agg_e @ wf1[:64] + agg_x @ wf1[64:]
  - agg_e (the segment-summed edge embeddings) is batch-independent and
    computed on host from the structural buffers (edge_attr / edge_rec).
    For HEALPix nested ordering (edge_attr = i%4, edge_rec = i//4) every
    receiver sees the same 4 embeddings, so agg_e @ wf1[:64] + bf1
    collapses to a single per-channel bias vector.
  - agg_x is a sum of each receiver's children rows of x.  With nested
    ordering each coarse pixel's 4 children are contiguous, so it's a
    fixed stride-4 group sum — no scatter needed.

The problem is memory-bound (30 MB/core HBM traffic in fp32 vs ~25 us
of engine work), so the fast path runs the whole pipeline in bf16
(tolerance is 2e-2; measured end-to-end error 4.1e-3): x is cast to
bf16 on host (halves input traffic), both matmuls run in bf16 (1
cycle/row instead of fp32's 4), and the output is written as bf16 and
upcast on host (halves output traffic).  Total 15.7 MB/core; measured
within ~7% of a DMA-only program moving the same bytes.

Layout: on host, xT is child-regrouped — within each 2048-column
super-tile block the 4 children of the 512 receivers are stored as
four contiguous 512-column slices — so every device-side operand is
unit-stride (full-rate PE moving reads; strided bf16 reads run at 1/4
rate and strided DVE ops lose the 16-bit 2x mode).

Device pipeline (per 512-receiver super-tile, transposed layout with
features on partitions so matmuls need no on-chip transposes):
    DMA xT chunk (128, 4096 bf16 = 1 MB) on SP HWDGE — finer chunks
        beat 2 MB ones once xin buffering is deep enough (4 bufs): the
        consumer waits whole-chunk DMA completion, so granularity wins
    TensorE: 4 accumulating bf16 matmuls fold the 4-child sum into the
        wf1x contraction (PSUM fp32 accumulate — frees VectorE and is
        more accurate than a bf16 tree sum)
    ScalarE: gelu(+folded bias) -> h bf16
    TensorE: psum2 = wf2-contract
    VectorE: tensor_scalar_add(+bf2) downcast -> bf16 obuf
    DMA out chunk (128, 2048 bf16) on Activation HWDGE (separate queue
        from inputs: measured -3 us vs sharing SP)
The last batch's input chunks taper (2,...,2,1,1 super-tiles) to
shorten the pipeline drain, and all constants load in 2 packed DMAs
(wp/bp) to shorten the cold-start preamble.

Sharding: receivers split uniformly across the 8 cores; both batches are
processed by every core (output rows B*R/8 per core).  Input x is
pre-transposed on host to (128, E) per batch so every DMA is dense.

Irregular edge_rec values (sorted, variable children counts) are handled
by a separate fp32 program via per-super-tile "layers": each layer
contributes up to 4 children per receiver, padded with zero columns
(host gather), and accumulates into the same PSUM tile.
"""

import numpy as np
import ml_dtypes

import concourse.bacc as bacc
import concourse.mybir as mybir
import concourse.tile as tile
from concourse.bass_utils import run_bass_kernel_spmd

# Problem constants (hardcoded per harness contract)
B = 2
E = 196608
R = 49152
F_IN = 128
EMBED = 64
NCORES = 8
RC = R // NCORES          # receivers per core (6144)
ST = 512                  # receivers per super-tile
NT = RC // ST             # super-tiles per core per batch (12)
CHUNK = 4 * ST            # x columns per super-tile (2048)

F32 = mybir.dt.float32
BF16 = mybir.dt.bfloat16
AF = mybir.ActivationFunctionType
NPBF16 = ml_dtypes.bfloat16

_prog_cache = {}


def _gelu_tanh(x):
    x = x.astype(np.float64)
    return 0.5 * x * (1.0 + np.tanh(np.sqrt(2.0 / np.pi) * (x + 0.044715 * x**3)))


def _build_fast(repeats=1, in_tiles=2, out_tiles=4, xin_bufs=4, work_bufs=4,
                psum_bufs=4, sum_mode="pe", ident_on="dve", act_merge=1,
                unroll=1, taper=True, out_eng="scalar", out_taper=False,
                in_split=False, psum2_bufs=None, obuf_bufs=3,
                head_taper=False, mixed=None):
    """bf16 fast path for the uniform HEALPix structure.

    Expects the child-regrouped host layout: within each 2048-column
    super-tile block of xT, columns are ordered [child j=0 for all 512
    receivers | j=1 | j=2 | j=3], so every child slice is unit-stride.

    in_tiles: super-tiles covered per input DMA (DMA = in_tiles*CHUNK bf16
        cols).
    out_tiles: super-tiles batched per output DMA.
    sum_mode: "pe"  — fold the 4-child sum into the first matmul as 4
        accumulating contiguous matmuls (PSUM fp32 accumulate, frees DVE);
        "dve" — 3 unit-stride bf16 adds (2x mode) on VectorE + 1 matmul.
    ident_on: engine for the final bias-add/downcast: "dve" (tensor_scalar
        on VectorE) or "act" (Identity activation on ScalarE).
    """
    if psum2_bufs is None:
        psum2_bufs = psum_bufs
    nc = bacc.Bacc(None, target_bir_lowering=False)
    ncols = NT * CHUNK
    xts = [
        nc.dram_tensor(f"xt{b}", [128, ncols], BF16, kind="ExternalInput")
        for b in range(B)
    ]
    wp = nc.dram_tensor("wp", [128, 256], BF16, kind="ExternalInput")
    bp = nc.dram_tensor("bp", [128, 2], F32, kind="ExternalInput")
    outt = nc.dram_tensor("outt", [128, B * RC], BF16, kind="ExternalOutput")

    with tile.TileContext(nc) as tc:
        with (
            tc.tile_pool(name="consts", bufs=1) as consts,
            tc.tile_pool(name="xin", bufs=xin_bufs) as xin,
            tc.tile_pool(name="work", bufs=work_bufs) as work,
            tc.tile_pool(name="obuf", bufs=obuf_bufs) as obuf,
            tc.tile_pool(name="psum", bufs=psum_bufs, space="PSUM") as psum,
            tc.tile_pool(name="psum2", bufs=psum2_bufs, space="PSUM") as psum2,
        ):
            # packed constants: 2 DMAs instead of 8 (shorter cold preamble)
            wp_sb = consts.tile([128, 256], BF16)
            nc.sync.dma_start(wp_sb[:], wp[:])
            bp_sb = consts.tile([128, 2], F32)
            nc.sync.dma_start(bp_sb[:], bp[:])
            w1_sb = wp_sb[:, 0:128]
            w2_sb = wp_sb[:, 128:256]
            b1_sb = bp_sb[:, 0:1]
            b2_sb = bp_sb[:, 1:2]

            g = act_merge
            assert NT % g == 0 and out_tiles % g == 0
            oeng = {"scalar": nc.scalar, "pool": nc.gpsimd,
                    "sync": nc.sync}[out_eng]

            def in_groups(b):
                # chunk sizes (in super-tiles) for batch b; tapering the tail
                # of the last batch shortens the pipeline drain, tapering the
                # head of the first batch starts compute earlier after fill
                if mixed is not None:
                    gs = list(mixed[b])
                    assert sum(gs) == NT, gs
                    return gs
                head = [1, 1, 2] if (head_taper and b == 0) else []
                tail = [2, 1, 1] if (taper and b == B - 1) else []
                rem = NT - sum(head) - sum(tail)
                gs = []
                while rem >= in_tiles:
                    gs.append(in_tiles)
                    rem -= in_tiles
                if rem:
                    gs.append(rem)
                gs = head + gs + tail
                assert sum(gs) == NT, gs
                return gs

            if out_taper:
                assert g == 1 and out_tiles == 4 and NT == 12

            def out_groups(b):
                if out_taper and b == B - 1:
                    return [out_tiles] * ((NT - 4) // out_tiles) + [2, 1, 1]
                return [out_tiles] * (NT // out_tiles)

            def body():
                for b in range(B):
                    col = 0
                    chunk = None
                    gsched = in_groups(b)
                    gidx = 0
                    goff = 0
                    fsched = out_groups(b)
                    fidx = 0
                    fstart = 0
                    flen = fsched[0]
                    ps1 = None
                    ps2 = None
                    h = None
                    ob = None
                    ieng = nc.scalar if (in_split and b == 1) else nc.sync
                    for k in range(NT):
                        if k == goff:
                            gl = gsched[gidx]
                            w = gl * CHUNK
                            chunk = xin.tile([128, w], BF16, tag="chunk")
                            ieng.dma_start(
                                chunk[:], xts[b][:, col : col + w]
                            )
                            col += w
                            goff += gl
                            gidx += 1
                        cs = chunk[:, (k - (goff - gl)) * CHUNK
                                   : (k - (goff - gl) + 1) * CHUNK]
                        if k % g == 0:
                            ps1 = psum.tile([128, g * ST], F32)
                        half = (k % g) * ST
                        p1 = ps1[:, half : half + ST]
                        if sum_mode == "dve":
                            # unit-stride pairwise sums (DVE 16-bit 2x mode)
                            u1 = work.tile([128, ST], BF16)
                            nc.vector.tensor_add(
                                u1[:], cs[:, 0:ST], cs[:, ST : 2 * ST]
                            )
                            u2 = work.tile([128, ST], BF16)
                            nc.vector.tensor_add(
                                u2[:], cs[:, 2 * ST : 3 * ST], cs[:, 3 * ST :]
                            )
                            agg = work.tile([128, ST], BF16)
                            nc.vector.tensor_add(agg[:], u1[:], u2[:])
                            nc.tensor.matmul(
                                p1, w1_sb, agg[:], start=True, stop=True
                            )
                        elif sum_mode == "hybrid":
                            # DVE folds child pairs, PE accumulates the rest
                            u1 = work.tile([128, ST], BF16)
                            nc.vector.tensor_add(
                                u1[:], cs[:, 0:ST], cs[:, ST : 2 * ST]
                            )
                            u2 = work.tile([128, ST], BF16)
                            nc.vector.tensor_add(
                                u2[:], cs[:, 2 * ST : 3 * ST], cs[:, 3 * ST :]
                            )
                            nc.tensor.matmul(
                                p1, w1_sb, u1[:], start=True, stop=False
                            )
                            nc.tensor.matmul(
                                p1, w1_sb, u2[:], start=False, stop=True
                            )
                        else:
                            # 4-child sum folded into the matmul: 4 contiguous
                            # moving slices accumulate into the same PSUM
                            for j in range(4):
                                nc.tensor.matmul(
                                    p1, w1_sb,
                                    cs[:, j * ST : (j + 1) * ST],
                                    start=(j == 0), stop=(j == 3),
                                )
                        if k % g == g - 1:
                            h = work.tile([128, g * ST], BF16)
                            nc.scalar.activation(
                                h[:], ps1[:], AF.Gelu_apprx_tanh, bias=b1_sb
                            )
                            ps2 = psum2.tile([128, g * ST], F32)
                            for m in range(g):
                                nc.tensor.matmul(
                                    ps2[:, m * ST : (m + 1) * ST], w2_sb,
                                    h[:, m * ST : (m + 1) * ST],
                                    start=True, stop=True,
                                )
                            if k - g + 1 == fstart:
                                flen = fsched[fidx]
                                ob = obuf.tile([128, flen * ST], BF16,
                                               tag="ob")
                            jo = (k - g + 1 - fstart) * ST
                            if ident_on == "dve":
                                nc.vector.tensor_scalar_add(
                                    ob[:, jo : jo + g * ST], ps2[:], b2_sb
                                )
                            else:
                                nc.scalar.activation(
                                    ob[:, jo : jo + g * ST], ps2[:], AF.Identity,
                                    bias=b2_sb,
                                )
                            if k == fstart + flen - 1:
                                off = b * RC + fstart * ST
                                oeng.dma_start(
                                    outt[:, off : off + flen * ST], ob[:]
                                )
                                fstart += flen
                                fidx += 1

            if repeats == 1:
                for _ in range(unroll):
                    body()
            else:
                with tc.For_i(0, repeats, 1):
                    for _ in range(unroll):
                        body()
    nc.compile()
    return nc


def _build_general(layer_counts, use_ct, repeats=1, in_tiles=2, out_tiles=4,
                   xin_bufs=3, work_bufs=4, psum_bufs=4):
    """fp32 general path: irregular sorted edge_rec / edge_attr.

    layer_counts: tuple of NT ints — number of 2048-column layer chunks
        feeding each super-tile (1 in the uniform HEALPix case).
    use_ct: if True, a per-receiver (128, RC) pre-GELU additive term is
        shipped and added before the activation (irregular edge_attr);
        otherwise a single per-channel bias vector suffices.
    """
    nc = bacc.Bacc(None, target_bir_lowering=False)
    ncols = sum(w * CHUNK for w in layer_counts)
    xts = [
        nc.dram_tensor(f"xt{b}", [128, ncols], F32, kind="ExternalInput")
        for b in range(B)
    ]
    w1 = nc.dram_tensor("w1", [128, 128], F32, kind="ExternalInput")
    w2 = nc.dram_tensor("w2", [128, 128], F32, kind="ExternalInput")
    b1 = nc.dram_tensor("b1", [128, 1], F32, kind="ExternalInput")
    b2 = nc.dram_tensor("b2", [128, 1], F32, kind="ExternalInput")
    if use_ct:
        ct = nc.dram_tensor("ct", [128, RC], F32, kind="ExternalInput")
    outt = nc.dram_tensor("outt", [128, B * RC], F32, kind="ExternalOutput")

    uniform_struct = all(w == 1 for w in layer_counts)
    if not uniform_struct:
        in_tiles = 1

    with tile.TileContext(nc) as tc:
        with (
            tc.tile_pool(name="consts", bufs=1) as consts,
            tc.tile_pool(name="xin", bufs=xin_bufs) as xin,
            tc.tile_pool(name="work", bufs=work_bufs) as work,
            tc.tile_pool(name="obuf", bufs=3) as obuf,
            tc.tile_pool(name="psum", bufs=psum_bufs, space="PSUM") as psum,
        ):
            w1_sb = consts.tile([128, 128], F32)
            nc.sync.dma_start(w1_sb[:], w1[:])
            w2_sb = consts.tile([128, 128], F32)
            nc.sync.dma_start(w2_sb[:], w2[:])
            b1_sb = consts.tile([128, 1], F32)
            nc.sync.dma_start(b1_sb[:], b1[:])
            b2_sb = consts.tile([128, 1], F32)
            nc.sync.dma_start(b2_sb[:], b2[:])
            if use_ct:
                ct_sb = consts.tile([128, RC], F32)
                nc.sync.dma_start(ct_sb[:], ct[:])

            def body():
                for b in range(B):
                    col = 0
                    chunk = None
                    ob = None
                    for k, w in enumerate(layer_counts):
                        ps1 = psum.tile([128, ST], F32)
                        for layer in range(w):
                            if uniform_struct:
                                if k % in_tiles == 0:
                                    chunk = xin.tile([128, in_tiles * CHUNK], F32)
                                    nc.sync.dma_start(
                                        chunk[:],
                                        xts[b][:, col : col + in_tiles * CHUNK],
                                    )
                                    col += in_tiles * CHUNK
                                j = (k % in_tiles) * CHUNK
                                cs = chunk[:, j : j + CHUNK]
                            else:
                                chunk = xin.tile([128, CHUNK], F32)
                                nc.sync.dma_start(
                                    chunk[:], xts[b][:, col : col + CHUNK]
                                )
                                col += CHUNK
                                cs = chunk[:]
                            # pairwise tree sum over groups of 4 adjacent cols
                            xp = cs.rearrange("p (n two) -> p n two", two=2)
                            u = work.tile([128, CHUNK // 2], F32)
                            nc.vector.tensor_add(u[:], xp[:, :, 0], xp[:, :, 1])
                            up = u[:].rearrange("p (n two) -> p n two", two=2)
                            agg = work.tile([128, ST], F32)
                            nc.vector.tensor_add(agg[:], up[:, :, 0], up[:, :, 1])
                            nc.tensor.matmul(
                                ps1[:], w1_sb[:], agg[:],
                                start=(layer == 0), stop=(layer == w - 1),
                            )
                        h = work.tile([128, ST], F32)
                        if use_ct:
                            tmp = work.tile([128, ST], F32)
                            nc.vector.tensor_add(
                                tmp[:], ps1[:], ct_sb[:, k * ST : (k + 1) * ST]
                            )
                            nc.scalar.activation(h[:], tmp[:], AF.Gelu_apprx_tanh)
                        else:
                            nc.scalar.activation(
                                h[:], ps1[:], AF.Gelu_apprx_tanh, bias=b1_sb[:]
                            )
                        ps2 = psum.tile([128, ST], F32)
                        nc.tensor.matmul(ps2[:], w2_sb[:], h[:], start=True, stop=True)
                        if k % out_tiles == 0:
                            ob = obuf.tile([128, out_tiles * ST], F32)
                        jo = (k % out_tiles) * ST
                        osl = ob[:, jo : jo + ST]
                        nc.scalar.activation(osl, ps2[:], AF.Identity, bias=b2_sb[:])
                        if k % out_tiles == out_tiles - 1:
                            off = b * RC + (k - out_tiles + 1) * ST
                            nc.sync.dma_start(
                                outt[:, off : off + out_tiles * ST], ob[:]
                            )

            if repeats == 1:
                body()
            else:
                with tc.For_i(0, repeats, 1):
                    body()
    nc.compile()
    return nc


def build_program(key, repeats=1):
    """key: ("fast",) or ("general", layer_counts, use_ct)."""
    ck = key + (repeats,)
    if ck not in _prog_cache:
        if key[0] == "fast":
            _prog_cache[ck] = _build_fast(repeats=repeats)
        else:
            _prog_cache[ck] = _build_general(key[1], key[2], repeats=repeats)
    return _prog_cache[ck]


def plan(**inputs):
    """Host-side prep: returns (nc, in_maps, assemble, key) where assemble
    maps per-core result dicts to the full output array."""
    x = np.asarray(inputs["x"], dtype=np.float32)
    edge_attr = np.asarray(inputs["edge_attr"], dtype=np.float32).reshape(-1)
    edge_rec = np.asarray(inputs["edge_rec"]).astype(np.int64)
    we1 = np.asarray(inputs["we1"], dtype=np.float32)
    be1 = np.asarray(inputs["be1"], dtype=np.float32)
    we2 = np.asarray(inputs["we2"], dtype=np.float32)
    be2 = np.asarray(inputs["be2"], dtype=np.float32)
    wf1 = np.asarray(inputs["wf1"], dtype=np.float32)
    bf1 = np.asarray(inputs["bf1"], dtype=np.float32)
    wf2 = np.asarray(inputs["wf2"], dtype=np.float32)
    bf2 = np.asarray(inputs["bf2"], dtype=np.float32)

    assert x.shape == (B, E, F_IN) and edge_rec.shape == (E,)

    # ---- host: structural analysis of the graph buffers -------------------
    uniform = np.array_equal(edge_rec, np.arange(E) // 4) and np.array_equal(
        edge_attr, (np.arange(E) % 4).astype(np.float32)
    )

    if uniform:
        # e-MLP contribution folded per receiver (batch-independent) into a
        # single per-channel pre-GELU bias:
        attr4 = np.arange(4, dtype=np.float64).reshape(4, 1)
        e4 = _gelu_tanh(attr4 @ we1.astype(np.float64) + be1) @ we2.astype(
            np.float64
        ) + be2.astype(np.float64)
        esum = e4.sum(axis=0)  # (64,)
        b1_eff = (
            bf1.astype(np.float64) + esum @ wf1[:EMBED].astype(np.float64)
        ).astype(np.float32)

        # ---- bf16 fast path ------------------------------------------------
        # Transpose to features-on-partitions AND regroup each 2048-column
        # super-tile block as [child0 x512 | child1 | child2 | child3] so
        # every device-side child slice is unit-stride (full-rate PE moving
        # reads / DVE 2x mode).  Column e = 4r+j lands at j*512 + (r%512).
        xT = np.ascontiguousarray(
            x.astype(NPBF16)
            .transpose(0, 2, 1)
            .reshape(B, F_IN, E // CHUNK, ST, 4)
            .transpose(0, 1, 2, 4, 3)
            .reshape(B, F_IN, E)
        )  # (B, 128, E) bf16, child-regrouped
        epc = E // NCORES
        wp = np.concatenate(
            [wf1[EMBED:].astype(NPBF16), wf2.astype(NPBF16)], axis=1
        )  # (128, 256) bf16: [wf1x | wf2], K=f_in on rows
        bpk = np.stack([b1_eff, bf2], axis=1).astype(np.float32)  # (128, 2)
        key = ("fast",)
        nc = build_program(key)
        in_maps = []
        for c in range(NCORES):
            in_maps.append(
                {
                    "xt0": np.ascontiguousarray(xT[0, :, c * epc : (c + 1) * epc]),
                    "xt1": np.ascontiguousarray(xT[1, :, c * epc : (c + 1) * epc]),
                    "wp": wp,
                    "bp": bpk,
                }
            )

        def assemble(results):
            out = np.empty((B, R, F_IN), dtype=np.float32)
            for c in range(NCORES):
                ot = results[c]["outt"].astype(np.float32)  # (128, B*RC)
                for b in range(B):
                    out[b, c * RC : (c + 1) * RC] = ot[:, b * RC : (b + 1) * RC].T
            return out

        return nc, in_maps, assemble, key

    # ---- fp32 general path ------------------------------------------------
    order = np.argsort(edge_rec, kind="stable")
    if np.array_equal(order, np.arange(E)):
        order = None
    er = edge_rec if order is None else edge_rec[order]
    ea = edge_attr if order is None else edge_attr[order]
    counts = np.bincount(er, minlength=R)
    starts = np.zeros(R + 1, dtype=np.int64)
    np.cumsum(counts, out=starts[1:])
    # host fold of the edge-embedding MLP (buffers only; no x involved)
    e = _gelu_tanh(ea.reshape(-1, 1) @ we1.astype(np.float64) + be1) @ we2.astype(
        np.float64
    ) + be2.astype(np.float64)
    cs = np.vstack([np.zeros((1, EMBED)), np.cumsum(e, axis=0)])
    agg_e = cs[starts[1:]] - cs[starts[:-1]]  # (R, 64)
    pre_bias = agg_e @ wf1[:EMBED].astype(np.float64) + bf1.astype(np.float64)
    pre_bias = pre_bias.astype(np.float32)  # (R, 128)
    if np.all(pre_bias == pre_bias[0]):
        b1_eff = pre_bias[0].copy()
        ct_full = None
    else:
        b1_eff = None
        ct_full = np.ascontiguousarray(pre_bias.T)  # (128, R)
    wmax = max(1, int(np.ceil(counts.max() / 4))) if E else 1
    layer_counts = (wmax,) * NT
    use_ct = ct_full is not None

    # padded gather: per super-tile, per layer, 4 child slots per receiver
    xT = np.ascontiguousarray(x.transpose(0, 2, 1))  # (B, 128, E)
    ncols = sum(w * CHUNK for w in layer_counts)
    idx = np.full((NCORES, ncols), E, dtype=np.int64)
    w0 = layer_counts[0]
    for c in range(NCORES):
        base = 0
        for k in range(NT):
            r0 = c * RC + k * ST
            for layer in range(w0):
                for j in range(4):
                    child = 4 * layer + j
                    rr = np.arange(r0, r0 + ST)
                    sel = starts[rr] + child
                    valid = sel < starts[rr + 1]
                    colpos = base + np.arange(ST) * 4 + j
                    idx[c, colpos[valid]] = sel[valid]
                base += CHUNK
    if order is not None:
        # map sorted-edge position -> original edge row in x
        ext = np.concatenate([order, [E]])
        idx = ext[idx]
    xT_ext = np.concatenate([xT, np.zeros((B, 128, 1), np.float32)], axis=2)
    core_x = [
        [np.take(xT_ext[b], idx[c], axis=1) for b in range(B)]
        for c in range(NCORES)
    ]

    w1x = np.ascontiguousarray(wf1[EMBED:])  # (128, 128), K=f_in on rows
    key = ("general", layer_counts, use_ct)
    nc = build_program(key)

    in_maps = []
    for c in range(NCORES):
        m = {
            "xt0": np.ascontiguousarray(core_x[c][0]),
            "xt1": np.ascontiguousarray(core_x[c][1]),
            "w1": w1x,
            "w2": wf2,
            "b2": bf2.reshape(128, 1),
        }
        if use_ct:
            m["ct"] = np.ascontiguousarray(ct_full[:, c * RC : (c + 1) * RC])
            m["b1"] = np.zeros((128, 1), np.float32)
        else:
            m["b1"] = b1_eff.reshape(128, 1)
        in_maps.append(m)

    def assemble(results):
        out = np.empty((B, R, F_IN), dtype=np.float32)
        for c in range(NCORES):
            ot = results[c]["outt"]  # (128, B*RC)
            for b in range(B):
                out[b, c * RC : (c + 1) * RC] = ot[:, b * RC : (b + 1) * RC].T
        return out

    return nc, in_maps, assemble, key


def kernel(**inputs) -> np.ndarray:
    nc, in_maps, assemble, _key = plan(**inputs)
    res = run_bass_kernel_spmd(nc, in_maps, core_ids=list(range(NCORES)))
    kernel.last_results = res
    return assemble(res.results)
